# revision 14
# baseline (speedup 1.0000x reference)
"""Trainium2 Bass kernel for a causal dense-transformer attention layer.

Reference computation (b=4, s=2048, d=1024, 16 heads, dh=64):
  qkv = x0 @ W_in ; causal softmax attention ; out = attn @ W_o
  y = LayerNorm(out + x0)   (no affine, eps=1e-5)

Sharding over 8 cores: core = (batch bi = core//2, head-group tp = core%2).
Each core computes QKV projection + attention for its 8 heads of one batch
(tensor parallel over head groups), then an AllToAll within the (bi) pair
re-shards from (heads-half, full seq) to (all heads, seq-half) so the output
projection + residual + LayerNorm run fully local, with no all-reduce.

On-chip layout: scores are computed transposed (keys on partitions, queries
on the free axis) so attn @ V needs no transposes; the softmax denominator
comes from an extra ones-column matmul packed into spare PE column groups.
Projections run in float32r (full PE rate at N=512), attention and output
projection in fp16 with fp32 PSUM accumulation. Softmax skips the max
subtraction (logits are O(5) here, exp is safe in fp32/fp16 range), and the
causal mask is applied multiplicatively after exp, which is exact.
"""

import os
import sys
from contextlib import ExitStack

import numpy as np

for _p in ("/opt/trn_rl_repo",):
    if os.path.isdir(_p) and _p not in sys.path:
        sys.path.insert(0, _p)

import concourse.bass as bass
import concourse.tile as tile
from concourse import bacc
from concourse import mybir
from concourse.bass_utils import run_bass_kernel_spmd

B, S, D = 4, 2048, 1024
NH, DH = 16, 64
HL = NH // 2          # heads per core
SH = S // 2           # output seq rows per core
NCORES = 8
SCALE = DH ** -0.5    # 0.125
LN_EPS = 1e-5

F32R = mybir.dt.float32r
F16 = mybir.dt.float16
F32 = mybir.dt.float32
Exp = mybir.ActivationFunctionType.Exp
Sqrt = mybir.ActivationFunctionType.Sqrt


def build_nc():
    nc = bacc.Bacc("TRN2", target_bir_lowering=False, num_devices=NCORES)
    xT = nc.declare_dram_parameter("xT", [D, S], F16, isOutput=False)
    wqk = nc.declare_dram_parameter("wqk", [D, 2 * HL * DH], F16, isOutput=False)
    wv = nc.declare_dram_parameter("wv", [D, HL * DH], F16, isOutput=False)
    wo = nc.declare_dram_parameter("wo", [HL * DH, D], F16, isOutput=False)
    xres = nc.declare_dram_parameter("xres", [SH, D], F32, isOutput=False)
    cmsk = nc.declare_dram_parameter("cmask", [128, 4 * 512], F16, isOutput=False)
    out = nc.declare_dram_parameter("out", [SH, D], F32, isOutput=True)

    with tile.TileContext(nc, num_cores=NCORES) as tc, ExitStack() as top:
        persist = top.enter_context(tc.tile_pool(name="persist", bufs=1))
        # QT rows 0..511 (tiles 0-3, head pair t on tile t), KT rows 512..1023
        qkt = [persist.tile([128, S], F16, name=f"qkt{m}") for m in range(8)]
        # V in (seq-part, head*dh free) orientation, 16 seq tiles
        vsb = [persist.tile([128, HL * DH], F16, name=f"vsb{m}") for m in range(16)]
        # attn-out^T (head*dh on partitions, seq free)
        aot = [persist.tile([128, S], F16, name=f"aot{t}") for t in range(4)]
        cm = persist.tile([128, 4 * 512], F16, name="cm")
        ones = persist.tile([128, 1], F16, name="ones")
        eps_t = persist.tile([128, 1], F32, name="eps_t")
        nc.vector.memset(ones, 1.0)
        nc.vector.memset(eps_t, LN_EPS)
        nc.sync.dma_start(out=cm, in_=cmsk[:, :])

        proj_ctx = ExitStack()
        proj_in = proj_ctx.enter_context(tc.tile_pool(name="proj_in", bufs=1))
        pjps = proj_ctx.enter_context(tc.tile_pool(name="pjps", bufs=2, space="PSUM"))
        xt = [proj_in.tile([128, S], F16, name=f"xt{k}") for k in range(8)]
        wqs = [proj_in.tile([128, 2 * HL * DH], F16, name=f"wqs{k}") for k in range(8)]
        wvs = [proj_in.tile([128, HL * DH], F16, name=f"wvs{k}") for k in range(8)]
        for k in range(8):
            nc.sync.dma_start(out=xt[k], in_=xT[k * 128:(k + 1) * 128, :])
            nc.sync.dma_start(out=wqs[k], in_=wqk[k * 128:(k + 1) * 128, :])
            nc.sync.dma_start(out=wvs[k], in_=wv[k * 128:(k + 1) * 128, :])

        def proj_v(m):
            ps = pjps.tile([128, 512], F32, tag="pj", name="pjv")
            for k in range(8):
                nc.tensor.matmul(ps, xt[k][:, m * 128:(m + 1) * 128], wvs[k],
                                 start=(k == 0), stop=(k == 7))
            nc.vector.tensor_copy(vsb[m], ps)

        def proj_qk(m):
            for q4 in range(4):
                ps = pjps.tile([128, 512], F32, tag="pj", name="pjqk")
                for k in range(8):
                    nc.tensor.matmul(ps, wqs[k][:, m * 128:(m + 1) * 128],
                                     xt[k][:, q4 * 512:(q4 + 1) * 512],
                                     start=(k == 0), stop=(k == 7))
                nc.vector.tensor_copy(qkt[m][:, q4 * 512:(q4 + 1) * 512], ps)

        # V first (attention consumes all V tiles by qc=3), then per-pair QK
        for m in range(16):
            proj_v(m)

        attn_ctx = ExitStack()
        adram = attn_ctx.enter_context(tc.tile_pool(name="adram", bufs=2, space="DRAM"))
        asb = attn_ctx.enter_context(tc.tile_pool(name="asb", bufs=6))
        scps = attn_ctx.enter_context(tc.tile_pool(name="scps", bufs=2, space="PSUM"))
        accps = attn_ctx.enter_context(tc.tile_pool(name="accps", bufs=1, space="PSUM"))
        small = attn_ctx.enter_context(tc.tile_pool(name="small", bufs=2))

        def attn_pair(t):
            q_t, k_t = qkt[t], qkt[4 + t]
            for qc in range(4):
                nkb = 4 * qc + 4
                qsl = slice(qc * 512, (qc + 1) * 512)
                av = accps.tile([128, 512], F32, tag="av", name="av")
                dn = accps.tile([128, 512], F32, tag="dn", name="dn")
                for kb in range(nkb):
                    ksl = slice(kb * 128, (kb + 1) * 128)
                    s0 = scps.tile([128, 512], F32, tag="s0", name="s0")
                    s1 = scps.tile([128, 512], F32, tag="s1", name="s1")
                    # scores^T = K^T.T @ Q^T, two heads row-packed (K=64 each)
                    nc.tensor.matmul(s0, k_t[0:64, ksl], q_t[0:64, qsl],
                                     start=True, stop=True)
                    nc.tensor.matmul(s1, k_t[64:128, ksl], q_t[64:128, qsl],
                                     start=True, stop=True)
                    e0 = asb.tile([128, 512], F16, tag="e0", name="e0")
                    e1 = asb.tile([128, 512], F16, tag="e1", name="e1")
                    nc.scalar.activation(e0, s0, Exp, scale=SCALE)
                    nc.scalar.activation(e1, s1, Exp, scale=SCALE)
                    r = kb - 4 * qc
                    if r >= 0:  # diagonal block: zero out masked entries
                        msl = slice(r * 512, (r + 1) * 512)
                        nc.vector.tensor_mul(e0, e0, cm[:, msl])
                        nc.vector.tensor_mul(e1, e1, cm[:, msl])
                    st, sp = (kb == 0), (kb == nkb - 1)
                    # attn-out^T accumulation, two heads col-packed into one
                    # PSUM bank on disjoint partition ranges. Each head runs
                    # its own start/stop accumulation group (has_written is
                    # per element); skip_group_check on the partition-offset
                    # ones — the simulator's global group tracker can't
                    # address base_partition != 0 slices.
                    nc.tensor.matmul(av[0:64, :], vsb[kb][:, (2 * t) * 64:(2 * t + 1) * 64],
                                     e0, start=st, stop=sp, tile_position=(0, 0))
                    nc.tensor.matmul(av[64:128, :], vsb[kb][:, (2 * t + 1) * 64:(2 * t + 2) * 64],
                                     e1, start=st, stop=sp, tile_position=(0, 64),
                                     skip_group_check=True)
                    # softmax denominators via ones-column, col groups 0 and 1
                    nc.tensor.matmul(dn[0:1, :], ones, e0,
                                     start=st, stop=sp, tile_position=(0, 0))
                    nc.tensor.matmul(dn[32:33, :], ones, e1,
                                     start=st, stop=sp, tile_position=(0, 32),
                                     skip_group_check=True)
                r0 = small.tile([1, 512], F32, tag="r0", name="r0")
                r1 = small.tile([1, 512], F32, tag="r1", name="r1")
                nc.vector.reciprocal(r0, dn[0:1, :])
                nc.vector.reciprocal(r1, dn[32:33, :])
                rb = small.tile([128, 512], F32, tag="rb", name="rb")
                rden = adram.tile([2, 512], F32, tag="rden", name="rden")
                nc.gpsimd.dma_start(out=rden[0:1, :], in_=r0)
                nc.gpsimd.dma_start(out=rden[1:2, :], in_=r1)
                for jh in range(2):
                    srow = rden[jh:jh + 1, :]
                    bc = bass.AP(tensor=srow.tensor, offset=srow.offset,
                                 ap=[[0, 64], [1, 512]])
                    nc.gpsimd.dma_start(out=rb[jh * 64:(jh + 1) * 64, :], in_=bc)
                nc.vector.tensor_mul(aot[t][:, qsl], av, rb)

        for t in range(4):
            proj_qk(t)      # Q rows for pair t
            proj_qk(4 + t)  # K rows for pair t
            attn_pair(t)

        attn_ctx.close()
        proj_ctx.close()

        fin = ExitStack()
        dpool = fin.enter_context(tc.tile_pool(name="dram", bufs=1, space="DRAM"))
        fsb = fin.enter_context(tc.tile_pool(name="fsb", bufs=1))
        fps = fin.enter_context(tc.tile_pool(name="fps", bufs=4, space="PSUM"))
        lnp = fin.enter_context(tc.tile_pool(name="lnp", bufs=3))

        # partial output projection over the FULL sequence with this core's
        # 512 head-dims; ReduceScatter over the pair then both sums the
        # head-group partials and hands each core its query half.
        wos = [fsb.tile([128, D], F16, name=f"wos{k}") for k in range(4)]
        xr = [fsb.tile([128, D], F32, name=f"xr{k}") for k in range(8)]
        for k in range(4):
            nc.sync.dma_start(out=wos[k], in_=wo[k * 128:(k + 1) * 128, :])
        for k in range(8):
            nc.sync.dma_start(out=xr[k], in_=xres[k * 128:(k + 1) * 128, :])

        rs_in = dpool.tile([S, D], F32, name="rs_in")
        rs_out = dpool.tile([SH, D], F32, name="rs_out")
        for m in range(16):
            pstg = lnp.tile([128, D], F32, tag="pstg", name="pstg")
            for n2 in range(2):
                po = fps.tile([128, 512], F32, tag="po", name="po")
                for k in range(4):
                    nc.tensor.matmul(po, aot[k][:, m * 128:(m + 1) * 128],
                                     wos[k][:, n2 * 512:(n2 + 1) * 512],
                                     start=(k == 0), stop=(k == 3))
                nc.vector.tensor_copy(pstg[:, n2 * 512:(n2 + 1) * 512], po)
            nc.sync.dma_start(out=rs_in[m * 128:(m + 1) * 128, :], in_=pstg)
        nc.gpsimd.collective_compute(
            "ReduceScatter", mybir.AluOpType.add,
            replica_groups=[[0, 1], [2, 3], [4, 5], [6, 7]],
            ins=[rs_in.opt()], outs=[rs_out.opt()])

        for m in range(8):
            y = lnp.tile([128, D], F32, tag="y", name="y")
            yin = lnp.tile([128, D], F32, tag="yin", name="yin")
            nc.sync.dma_start(out=yin, in_=rs_out[m * 128:(m + 1) * 128, :])
            nc.vector.tensor_add(y, yin, xr[m])
            stats = lnp.tile([128, 2, 6], F32, tag="st", name="st")
            mv = lnp.tile([128, 2], F32, tag="mv", name="mv")
            for sg in range(2):
                nc.vector.bn_stats(out=stats[:, sg, :], in_=y[:, sg * 512:(sg + 1) * 512])
            nc.vector.bn_aggr(out=mv, in_=stats)
            rstd = lnp.tile([128, 1], F32, tag="rs", name="rs")
            nc.scalar.activation(out=rstd, in_=mv[:, 1:2], func=Sqrt, bias=eps_t)
            nc.vector.reciprocal(rstd, rstd)
            ot = lnp.tile([128, D], F32, tag="ot", name="ot")
            nc.vector.tensor_scalar(out=ot, in0=y, scalar1=mv[:, 0:1], scalar2=rstd,
                                    op0=mybir.AluOpType.subtract,
                                    op1=mybir.AluOpType.mult)
            nc.sync.dma_start(out=out[m * 128:(m + 1) * 128, :], in_=ot)
        fin.close()
    nc.compile()
    return nc


def _build_cmask():
    k = np.arange(128)[:, None]
    q = np.arange(512)[None, :]
    blocks = [(r * 128 + k <= q).astype(np.float16) for r in range(4)]
    return np.concatenate(blocks, axis=1)


def _make_in_maps(x0, W_in, W_o):
    x0 = np.asarray(x0, np.float32)
    W_in = np.asarray(W_in, np.float32)
    W_o = np.asarray(W_o, np.float32)
    wo16 = W_o.astype(np.float16)
    cmask = _build_cmask()
    in_maps = []
    for core in range(NCORES):
        bi, half = core // 2, core % 2
        hs = range(half * HL, half * HL + HL)
        wqk = np.concatenate(
            [W_in[:, h * 3 * DH: h * 3 * DH + DH] for h in hs]
            + [W_in[:, h * 3 * DH + DH: h * 3 * DH + 2 * DH] for h in hs], axis=1)
        wv = np.concatenate(
            [W_in[:, h * 3 * DH + 2 * DH: h * 3 * DH + 3 * DH] for h in hs], axis=1)
        in_maps.append(dict(
            xT=np.ascontiguousarray(x0[bi].T).astype(np.float16),
            wqk=np.ascontiguousarray(wqk).astype(np.float16),
            wv=np.ascontiguousarray(wv).astype(np.float16),
            wo=np.ascontiguousarray(wo16[half * HL * DH:(half + 1) * HL * DH]),
            xres=np.ascontiguousarray(x0[bi, half * SH:(half + 1) * SH]),
            cmask=cmask))
    return in_maps


_NC = None


def _run(x0, W_in, W_o, **run_kwargs):
    global _NC
    if _NC is None:
        _NC = build_nc()
    in_maps = _make_in_maps(x0, W_in, W_o)
    return run_bass_kernel_spmd(_NC, in_maps, list(range(NCORES)), **run_kwargs)


def kernel(x0, W_in, W_o, src_mask=None):
    res = _run(x0, W_in, W_o).results
    out = np.empty((B, S, D), np.float32)
    for core in range(NCORES):
        bi, half = core // 2, core % 2
        out[bi, half * SH:(half + 1) * SH] = res[core]["out"]
    return out


# revision 17
# speedup vs baseline: 1.1385x; 1.1385x over previous
"""Trainium2 Bass kernel for a causal dense-transformer attention layer.

Reference computation (b=4, s=2048, d=1024, 16 heads, dh=64):
  qkv = x0 @ W_in ; causal softmax attention ; out = attn @ W_o
  y = LayerNorm(out + x0)   (no affine, eps=1e-5)

Sharding over 8 cores: core = (batch bi = core//2, head-group tp = core%2).
Each core computes QKV projection + attention for its 8 heads of one batch
(tensor parallel over head groups), then an AllToAll within the (bi) pair
re-shards from (heads-half, full seq) to (all heads, seq-half) so the output
projection + residual + LayerNorm run fully local, with no all-reduce.

On-chip layout: scores are computed transposed (keys on partitions, queries
on the free axis) so attn @ V needs no transposes; the softmax denominator
comes from an extra ones-column matmul packed into spare PE column groups.
Projections run in float32r (full PE rate at N=512), attention and output
projection in fp16 with fp32 PSUM accumulation. Softmax skips the max
subtraction (logits are O(5) here, exp is safe in fp32/fp16 range), and the
causal mask is applied multiplicatively after exp, which is exact.
"""

import os
import sys
from contextlib import ExitStack

import numpy as np

for _p in ("/opt/trn_rl_repo",):
    if os.path.isdir(_p) and _p not in sys.path:
        sys.path.insert(0, _p)

import concourse.bass as bass
import concourse.tile as tile
from concourse import bacc
from concourse import mybir
from concourse.bass_utils import run_bass_kernel_spmd

B, S, D = 4, 2048, 1024
NH, DH = 16, 64
HL = NH // 2          # heads per core
SH = S // 2           # output seq rows per core
NCORES = 8
SCALE = DH ** -0.5    # 0.125
LN_EPS = 1e-5

F32R = mybir.dt.float32r
F16 = mybir.dt.float16
F32 = mybir.dt.float32
Exp = mybir.ActivationFunctionType.Exp
Sqrt = mybir.ActivationFunctionType.Sqrt


def build_nc():
    nc = bacc.Bacc("TRN2", target_bir_lowering=False, num_devices=NCORES)
    xT = nc.declare_dram_parameter("xT", [D, S], F16, isOutput=False)
    wqk = nc.declare_dram_parameter("wqk", [D, 2 * HL * DH], F16, isOutput=False)
    wv = nc.declare_dram_parameter("wv", [D, HL * DH], F16, isOutput=False)
    wo = nc.declare_dram_parameter("wo", [HL * DH, D], F16, isOutput=False)
    xres = nc.declare_dram_parameter("xres", [SH, D], F32, isOutput=False)
    cmsk = nc.declare_dram_parameter("cmask", [128, 4 * 512], F16, isOutput=False)
    out = nc.declare_dram_parameter("out", [SH, D], F32, isOutput=True)

    with tile.TileContext(nc, num_cores=NCORES) as tc, ExitStack() as top:
        persist = top.enter_context(tc.tile_pool(name="persist", bufs=1))
        # QT rows 0..511 (tiles 0-3, head pair t on tile t), KT rows 512..1023
        qkt = [persist.tile([128, S], F16, name=f"qkt{m}") for m in range(8)]
        # V in (seq-part, head*dh free) orientation, 16 seq tiles
        vsb = [persist.tile([128, HL * DH], F16, name=f"vsb{m}") for m in range(16)]
        # attn-out^T (head*dh on partitions, seq free)
        aot = [persist.tile([128, S], F16, name=f"aot{t}") for t in range(4)]
        # unnormalized attn-out^T (fp32) + softmax denominators, normalized
        # in a deferred pass so the PSUM banks free up immediately
        aot_u = [persist.tile([128, S], F32, name=f"aotu{t}") for t in range(4)]
        cm = persist.tile([128, 4 * 512], F16, name="cm")
        ones = persist.tile([128, 1], F16, name="ones")
        eps_t = persist.tile([128, 1], F32, name="eps_t")
        nc.vector.memset(ones, 1.0)
        nc.vector.memset(eps_t, LN_EPS)
        nc.sync.dma_start(out=cm, in_=cmsk[:, :])

        proj_ctx = ExitStack()
        proj_in = proj_ctx.enter_context(tc.tile_pool(name="proj_in", bufs=1))
        pjps = proj_ctx.enter_context(tc.tile_pool(name="pjps", bufs=2, space="PSUM"))
        xt = [proj_in.tile([128, S], F16, name=f"xt{k}") for k in range(8)]
        wqs = [proj_in.tile([128, 2 * HL * DH], F16, name=f"wqs{k}") for k in range(8)]
        wvs = [proj_in.tile([128, HL * DH], F16, name=f"wvs{k}") for k in range(8)]
        for k in range(8):
            nc.sync.dma_start(out=xt[k], in_=xT[k * 128:(k + 1) * 128, :])
            nc.sync.dma_start(out=wqs[k], in_=wqk[k * 128:(k + 1) * 128, :])
            nc.sync.dma_start(out=wvs[k], in_=wv[k * 128:(k + 1) * 128, :])

        def proj_v(m):
            ps = pjps.tile([128, 512], F32, tag="pj", name="pjv")
            for k in range(8):
                nc.tensor.matmul(ps, xt[k][:, m * 128:(m + 1) * 128], wvs[k],
                                 start=(k == 0), stop=(k == 7))
            nc.vector.tensor_copy(vsb[m], ps)

        def proj_qk(m):
            for q4 in range(4):
                ps = pjps.tile([128, 512], F32, tag="pj", name="pjqk")
                for k in range(8):
                    nc.tensor.matmul(ps, wqs[k][:, m * 128:(m + 1) * 128],
                                     xt[k][:, q4 * 512:(q4 + 1) * 512],
                                     start=(k == 0), stop=(k == 7))
                nc.vector.tensor_copy(qkt[m][:, q4 * 512:(q4 + 1) * 512], ps)

        # V first (attention consumes all V tiles by qc=3), then per-pair QK
        for m in range(16):
            proj_v(m)

        attn_ctx = ExitStack()
        adram = attn_ctx.enter_context(tc.tile_pool(name="adram", bufs=2, space="DRAM"))
        asb = attn_ctx.enter_context(tc.tile_pool(name="asb", bufs=6))
        scps = attn_ctx.enter_context(tc.tile_pool(name="scps", bufs=2, space="PSUM"))
        accps = attn_ctx.enter_context(tc.tile_pool(name="accps", bufs=1, space="PSUM"))
        small = attn_ctx.enter_context(tc.tile_pool(name="small", bufs=2))

        def attn_pair(t):
            q_t, k_t = qkt[t], qkt[4 + t]
            for qc in range(4):
                nkb = 4 * qc + 4
                qsl = slice(qc * 512, (qc + 1) * 512)
                av = accps.tile([128, 512], F32, tag="av", name="av")
                dn = accps.tile([128, 512], F32, tag="dn", name="dn")
                for kb in range(nkb):
                    ksl = slice(kb * 128, (kb + 1) * 128)
                    s0 = scps.tile([128, 512], F32, tag="s0", name="s0")
                    s1 = scps.tile([128, 512], F32, tag="s1", name="s1")
                    # scores^T = K^T.T @ Q^T, two heads row-packed (K=64 each)
                    nc.tensor.matmul(s0, k_t[0:64, ksl], q_t[0:64, qsl],
                                     start=True, stop=True)
                    nc.tensor.matmul(s1, k_t[64:128, ksl], q_t[64:128, qsl],
                                     start=True, stop=True)
                    e0 = asb.tile([128, 512], F16, tag="e0", name="e0")
                    e1 = asb.tile([128, 512], F16, tag="e1", name="e1")
                    nc.scalar.activation(e0, s0, Exp, scale=SCALE)
                    nc.scalar.activation(e1, s1, Exp, scale=SCALE)
                    r = kb - 4 * qc
                    if r >= 0:  # diagonal block: zero out masked entries
                        msl = slice(r * 512, (r + 1) * 512)
                        nc.vector.tensor_mul(e0, e0, cm[:, msl])
                        nc.vector.tensor_mul(e1, e1, cm[:, msl])
                    st, sp = (kb == 0), (kb == nkb - 1)
                    # attn-out^T accumulation, two heads col-packed into one
                    # PSUM bank on disjoint partition ranges. Each head runs
                    # its own start/stop accumulation group (has_written is
                    # per element); skip_group_check on the partition-offset
                    # ones — the simulator's global group tracker can't
                    # address base_partition != 0 slices.
                    nc.tensor.matmul(av[0:64, :], vsb[kb][:, (2 * t) * 64:(2 * t + 1) * 64],
                                     e0, start=st, stop=sp, tile_position=(0, 0))
                    nc.tensor.matmul(av[64:128, :], vsb[kb][:, (2 * t + 1) * 64:(2 * t + 2) * 64],
                                     e1, start=st, stop=sp, tile_position=(0, 64),
                                     skip_group_check=True)
                    # softmax denominators via ones-column, col groups 0 and 1
                    nc.tensor.matmul(dn[0:1, :], ones, e0,
                                     start=st, stop=sp, tile_position=(0, 0))
                    nc.tensor.matmul(dn[32:33, :], ones, e1,
                                     start=st, stop=sp, tile_position=(0, 32),
                                     skip_group_check=True)
                # drain PSUM fast (frees av/dn for the next chunk), then
                # normalize asynchronously off the PE critical path
                nc.vector.tensor_copy(aot_u[t][:, qsl], av)
                r0 = small.tile([1, 512], F32, tag="r0", name="r0")
                r1 = small.tile([1, 512], F32, tag="r1", name="r1")
                nc.vector.tensor_copy(r0, dn[0:1, :])
                nc.vector.tensor_copy(r1, dn[32:33, :])
                nc.vector.reciprocal(r0, r0)
                nc.vector.reciprocal(r1, r1)
                rdend = adram.tile([2, 512], F32, tag="rdend", name="rdend")
                nc.gpsimd.dma_start(out=rdend[0:1, :], in_=r0)
                nc.gpsimd.dma_start(out=rdend[1:2, :], in_=r1)
                rb = small.tile([128, 512], F32, tag="rb", name="rb")
                for jh in range(2):
                    srow = rdend[jh:jh + 1, :]
                    bc = bass.AP(tensor=srow.tensor, offset=srow.offset,
                                 ap=[[0, 64], [1, 512]])
                    nc.gpsimd.dma_start(out=rb[jh * 64:(jh + 1) * 64, :], in_=bc)
                nc.vector.tensor_mul(aot[t][:, qsl], aot_u[t][:, qsl], rb)

        for t in range(4):
            proj_qk(t)      # Q rows for pair t
            proj_qk(4 + t)  # K rows for pair t
            attn_pair(t)

        attn_ctx.close()
        proj_ctx.close()

        fin = ExitStack()
        dpool = fin.enter_context(tc.tile_pool(name="dram", bufs=1, space="DRAM"))
        fsb = fin.enter_context(tc.tile_pool(name="fsb", bufs=1))
        fps = fin.enter_context(tc.tile_pool(name="fps", bufs=4, space="PSUM"))
        lnp = fin.enter_context(tc.tile_pool(name="lnp", bufs=3))

        # partial output projection over the FULL sequence with this core's
        # 512 head-dims; ReduceScatter over the pair then both sums the
        # head-group partials and hands each core its query half.
        wos = [fsb.tile([128, D], F16, name=f"wos{k}") for k in range(4)]
        xr = [fsb.tile([128, D], F32, name=f"xr{k}") for k in range(8)]
        for k in range(4):
            nc.sync.dma_start(out=wos[k], in_=wo[k * 128:(k + 1) * 128, :])
        for k in range(8):
            nc.sync.dma_start(out=xr[k], in_=xres[k * 128:(k + 1) * 128, :])

        # chunked fp16 ReduceScatter over the pair, overlapped with the
        # output projection: chunk c carries output rows [c*256, (c+1)*256)
        # of each query half; the collective hands each core its half.
        rs_in = [dpool.tile([512, D], F16, name=f"rs_in{c}", bufs=4) for c in range(4)]
        rs_out = [dpool.tile([256, D], F16, name=f"rs_out{c}", bufs=4) for c in range(4)]
        for c in range(4):
            for j, m in enumerate((2 * c, 2 * c + 1, 8 + 2 * c, 8 + 2 * c + 1)):
                pstg = lnp.tile([128, D], F16, tag="pstg", name="pstg")
                for n2 in range(2):
                    po = fps.tile([128, 512], F32, tag="po", name="po")
                    for k in range(4):
                        nc.tensor.matmul(po, aot[k][:, m * 128:(m + 1) * 128],
                                         wos[k][:, n2 * 512:(n2 + 1) * 512],
                                         start=(k == 0), stop=(k == 3))
                    nc.vector.tensor_copy(pstg[:, n2 * 512:(n2 + 1) * 512], po)
                nc.sync.dma_start(out=rs_in[c][j * 128:(j + 1) * 128, :], in_=pstg)
            nc.gpsimd.collective_compute(
                "ReduceScatter", mybir.AluOpType.add,
                replica_groups=[[0, 1], [2, 3], [4, 5], [6, 7]],
                ins=[rs_in[c].opt()], outs=[rs_out[c].opt()])

        for m in range(8):
            c, j = m // 2, m % 2
            y = lnp.tile([128, D], F32, tag="y", name="y")
            yin = lnp.tile([128, D], F16, tag="yin", name="yin")
            nc.sync.dma_start(out=yin, in_=rs_out[c][j * 128:(j + 1) * 128, :])
            nc.vector.tensor_add(y, yin, xr[m])
            stats = lnp.tile([128, 2, 6], F32, tag="st", name="st")
            mv = lnp.tile([128, 2], F32, tag="mv", name="mv")
            for sg in range(2):
                nc.vector.bn_stats(out=stats[:, sg, :], in_=y[:, sg * 512:(sg + 1) * 512])
            nc.vector.bn_aggr(out=mv, in_=stats)
            rstd = lnp.tile([128, 1], F32, tag="rs", name="rs")
            nc.scalar.activation(out=rstd, in_=mv[:, 1:2], func=Sqrt, bias=eps_t)
            nc.vector.reciprocal(rstd, rstd)
            ot = lnp.tile([128, D], F32, tag="ot", name="ot")
            nc.vector.tensor_scalar(out=ot, in0=y, scalar1=mv[:, 0:1], scalar2=rstd,
                                    op0=mybir.AluOpType.subtract,
                                    op1=mybir.AluOpType.mult)
            nc.sync.dma_start(out=out[m * 128:(m + 1) * 128, :], in_=ot)
        fin.close()
    nc.compile()
    return nc


def _build_cmask():
    k = np.arange(128)[:, None]
    q = np.arange(512)[None, :]
    blocks = [(r * 128 + k <= q).astype(np.float16) for r in range(4)]
    return np.concatenate(blocks, axis=1)


def _make_in_maps(x0, W_in, W_o):
    x0 = np.asarray(x0, np.float32)
    W_in = np.asarray(W_in, np.float32)
    W_o = np.asarray(W_o, np.float32)
    wo16 = W_o.astype(np.float16)
    cmask = _build_cmask()
    in_maps = []
    for core in range(NCORES):
        bi, half = core // 2, core % 2
        hs = range(half * HL, half * HL + HL)
        wqk = np.concatenate(
            [W_in[:, h * 3 * DH: h * 3 * DH + DH] for h in hs]
            + [W_in[:, h * 3 * DH + DH: h * 3 * DH + 2 * DH] for h in hs], axis=1)
        wv = np.concatenate(
            [W_in[:, h * 3 * DH + 2 * DH: h * 3 * DH + 3 * DH] for h in hs], axis=1)
        in_maps.append(dict(
            xT=np.ascontiguousarray(x0[bi].T).astype(np.float16),
            wqk=np.ascontiguousarray(wqk).astype(np.float16),
            wv=np.ascontiguousarray(wv).astype(np.float16),
            wo=np.ascontiguousarray(wo16[half * HL * DH:(half + 1) * HL * DH]),
            xres=np.ascontiguousarray(x0[bi, half * SH:(half + 1) * SH]),
            cmask=cmask))
    return in_maps


_NC = None


def _run(x0, W_in, W_o, **run_kwargs):
    global _NC
    if _NC is None:
        _NC = build_nc()
    in_maps = _make_in_maps(x0, W_in, W_o)
    return run_bass_kernel_spmd(_NC, in_maps, list(range(NCORES)), **run_kwargs)


def kernel(x0, W_in, W_o, src_mask=None):
    res = _run(x0, W_in, W_o).results
    out = np.empty((B, S, D), np.float32)
    for core in range(NCORES):
        bi, half = core // 2, core % 2
        out[bi, half * SH:(half + 1) * SH] = res[core]["out"]
    return out
